# revision 11
# baseline (speedup 1.0000x reference)
"""HarsanyiNet forward on 8 TRN2 NeuronCores (Bass/Tile).

Model (reference):
    harsanyi_block(x, v, fc):
        m = (v > 0)                                    # [O, I] mask
        delta = prod_i [ tanh(g*|x_i|) if m else 1 ]   # [B, O]
        h = relu((x @ (fc*m).T) * delta)
    y = h0 @ head0.T + h1 @ head1.T   (two blocks, h0 feeds block 1)

Key algebraic moves:
  * The [B, O, I] masked product becomes a matmul in log space:
        delta = exp(L @ m.T),  L[b,i] = log(tanh(g*|x[b,i]|))
    with log(tanh(y)) = ln(1-z) - ln(1+z), z = exp(-2*g*y), so the
    whole transcendental chain is {abs, exp, ln} — all in ONE ScalarE
    table set (natural_log_exp_and_others) -> a single table load.
  * Matmuls run on the bf16 PE path (4x the fp32 rate) with hi/lo
    split operands for fp32-grade accuracy.  The mask m is exact in
    bf16; fc and x are split on the host (w_hi = m*bf16_hi(fc) is
    exact because masking by 0/1 commutes with rounding); L is split
    on-device.  The z<=1-2^-24 clamp keeps Ln inputs positive, so
    every intermediate stays finite.
  * The serial DMA->abs->exp->min->ln->ln->sub->split chain is
    pipelined in two column halves so ScalarE and VectorE stages of
    half 0 overlap half 1.

Sharding: the output-hidden dim is split across the 8 cores, so each
core reads only 1/8 of v/fc per layer (~0.8 MB/core/launch instead of
16.4 MB replicated).  Layer 1 needs the full h0, which is bounced
through the host between two launches of the SAME compiled program
(an on-device AllGather costs ~80us in this environment, the host
bounce costs zero device time).  Partial head outputs are summed on
the host.

Layout: on-device tensors are feature-major [feature, batch]; the
1024-long feature dims are pre-split on the host into 8 chunk-major
blocks of 128 partitions, so every DMA is one dense [128, N] transfer
and every matmul operand slice is a natural column block.
"""
import sys

import numpy as np

sys.path.insert(0, "/opt/trn_rl_repo")

import ml_dtypes  # noqa: E402

from concourse import bacc, mybir, tile  # noqa: E402
from concourse.alu_op_type import AluOpType  # noqa: E402
from concourse.bass_utils import run_bass_kernel_spmd  # noqa: E402

B, NIN, HID, C = 64, 1024, 1024, 10
GAMMA = 100.0
N_CORES = 8
OSH = HID // N_CORES        # output-hidden rows per core (128)
KCH = NIN // 128            # contraction chunks (8)
KB = KCH * B                # activation columns, chunk-major (512)
KO = KCH * OSH              # weight columns, chunk-major (1024)
NH = 2                      # pipeline halves for the L chain
HB = KB // NH               # columns per half (256)
HCH = KCH // NH             # chunks per half (4)
# Upper clamp for z = exp(-2g|x|): keeps 1-z >= 2^-24 so Ln never sees 0
# (the reference's exact-zero delta becomes exp(-16.6)~3e-8 per factor,
# far below the output's scale).
LCLAMP = -30000.0
F32 = mybir.dt.float32
BF16 = mybir.dt.bfloat16
BF16_NP = ml_dtypes.bfloat16

PROFILE = {"enable": False, "trace_kwargs": {}, "runs": []}
_CACHE = {}


def _force_act_table_set(target="natural_log_exp_and_others"):
    """Make the act-table-load pass place every activation in `target`
    (it otherwise picks the first set per function, costing one ~2.7us
    table switch per transition Exp->Ln->Exp).  Indices of the table
    list are act_func_set_ids, so ordering is preserved and all other
    sets are emptied."""
    import concourse.bacc as bacc_mod
    from concourse.hw_specs import get_activation_tables as real_tabs

    def patched(arch):
        tabs = real_tabs(arch)
        return {name: (funcs if name == target else set())
                for name, funcs in tabs.items()}

    bacc_mod.get_activation_tables = patched


def _build():
    _force_act_table_set()
    nc = bacc.Bacc("TRN2", target_bir_lowering=False, debug=False,
                   num_devices=N_CORES, enable_asserts=False)
    xTf = nc.declare_dram_parameter("xTf", [128, KB], F32, isOutput=False)
    # bf16 hi/lo pairs packed side by side: [hi | lo]
    xhl = nc.declare_dram_parameter("xhl", [128, 2 * KB], BF16, isOutput=False)
    vT = nc.declare_dram_parameter("vT", [128, KO], BF16, isOutput=False)
    fhl = nc.declare_dram_parameter("fhl", [128, 2 * KO], BF16, isOutput=False)
    hdT = nc.declare_dram_parameter("hdT", [OSH, C], F32, isOutput=False)
    h_sh = nc.declare_dram_parameter("h_sh", [OSH, B], F32, isOutput=True)
    y_part = nc.declare_dram_parameter("y_part", [C, B], F32, isOutput=True)
    Act = mybir.ActivationFunctionType

    with tile.TileContext(nc) as tc:
        with (
            tc.tile_pool(name="sb", bufs=1) as sb,
            tc.tile_pool(name="ps", bufs=1, space="PSUM") as ps,
        ):
            # x (f32) arrives in NH column-halves so the L chain can start
            # on half 0 while half 1 is still in flight.
            xf = sb.tile([128, KB], F32)
            for hf in range(NH):
                nc.sync.dma_start(xf[:, hf * HB:(hf + 1) * HB],
                                  xTf[:, hf * HB:(hf + 1) * HB])
            vt = sb.tile([128, KO], BF16)
            nc.sync.dma_start(vt[:], vT[:, :])
            fb = sb.tile([128, 2 * KO], BF16)
            nc.sync.dma_start(fb[:], fhl[:, :])
            xb = sb.tile([128, 2 * KB], BF16)
            nc.sync.dma_start(xb[:], xhl[:, :])
            hdt = sb.tile([OSH, C], F32)
            nc.sync.dma_start(hdt[:], hdT[:, :])

            # L = log(tanh(g*|x|)) = ln(1-z) - ln(1+z), z = exp(-2g|x|),
            # pipelined over NH column halves.  The small negative bias on
            # the Exp input keeps z strictly below 1 (so Ln(1-z) is finite
            # for x = 0); the -30000 clamp catches -inf if the LUT rounds
            # z up to 1 anyway.
            a = sb.tile([128, KB], F32)
            z = sb.tile([128, KB], F32)
            p = sb.tile([128, KB], F32)
            q = sb.tile([128, KB], F32)
            Lh = sb.tile([128, KB], BF16)
            Ll = sb.tile([128, KB], BF16)
            L = sb.tile([128, KB], F32)
            eps = sb.tile([128, 1], F32)
            nc.vector.memset(eps[:], -1e-6)
            m = sb.tile([128, KO], BF16)
            w = sb.tile([128, 2 * KO], BF16)
            S = ps.tile([OSH, B], F32)
            HL = ps.tile([OSH, B], F32)
            n_s = 2 * KCH
            i_s = 0

            def s_mms(hf):
                nonlocal i_s
                for k in range(hf * HCH, (hf + 1) * HCH):
                    osl = slice(k * OSH, (k + 1) * OSH)
                    bsl = slice(k * B, (k + 1) * B)
                    for rhs in (Lh, Ll):
                        nc.tensor.matmul(S[:], m[:, osl], rhs[:, bsl],
                                         start=(i_s == 0),
                                         stop=(i_s == n_s - 1))
                        i_s += 1

            for hf in range(NH):
                cs = slice(hf * HB, (hf + 1) * HB)
                nc.vector.scalar_tensor_tensor(a[:, cs], xf[:, cs], -1.0,
                                               xf[:, cs],
                                               op0=AluOpType.mult,
                                               op1=AluOpType.max)
                nc.scalar.activation(z[:, cs], a[:, cs], Act.Exp,
                                     scale=-2.0 * GAMMA, bias=eps[:])
                nc.scalar.activation(p[:, cs], z[:, cs], Act.Ln,
                                     bias=1.0, scale=-1.0)
                nc.scalar.activation(q[:, cs], z[:, cs], Act.Ln,
                                     bias=1.0, scale=1.0)
                if hf == 0:
                    # m = (v > 0) as 0/1 (v is exactly +-1): slot into the
                    # DVE stream while ScalarE works on ln, right before
                    # the first L ops so S matmuls can start early.
                    nc.vector.tensor_scalar_max(m[:], vt[:], 0.0)
                nc.vector.scalar_tensor_tensor(L[:, cs], p[:, cs], LCLAMP,
                                               q[:, cs],
                                               op0=AluOpType.max,
                                               op1=AluOpType.subtract)
                nc.vector.tensor_copy(Lh[:, cs], L[:, cs])
                nc.vector.tensor_sub(Ll[:, cs], L[:, cs], Lh[:, cs])
                s_mms(hf)
                if hf == 0:
                    # w = fc * m per half, after half-0 L ops so the DVE
                    # doesn't head-of-line block on the fc DMA.
                    nc.vector.tensor_mul(w[:, :KO], m[:], fb[:, :KO])
                    nc.vector.tensor_mul(w[:, KO:], m[:], fb[:, KO:])

            # HL matmuls last: they wait on the (late) fc DMA anyway, and
            # keeping them off the in-order PE queue lets S finish early.
            n_hl = 3 * KCH
            i_hl = 0
            for k in range(KCH):
                osl = slice(k * OSH, (k + 1) * OSH)
                osl_lo = slice(KO + k * OSH, KO + (k + 1) * OSH)
                bsl = slice(k * B, (k + 1) * B)
                bsl_lo = slice(KB + k * B, KB + (k + 1) * B)
                # HL += w_hi.T x_hi + w_hi.T x_lo + w_lo.T x_hi
                for lsl, rsl in ((osl, bsl), (osl, bsl_lo), (osl_lo, bsl)):
                    nc.tensor.matmul(HL[:], w[:, lsl], xb[:, rsl],
                                     start=(i_hl == 0),
                                     stop=(i_hl == n_hl - 1))
                    i_hl += 1

            # h = relu(HL) * exp(S)   (= relu(HL*exp(S)) since exp(S) > 0;
            # the relu runs as soon as HL closes, in parallel with exp)
            d = sb.tile([OSH, B], F32)
            nc.scalar.activation(d[:], S[:], Act.Exp)
            hr = sb.tile([OSH, B], F32)
            nc.vector.tensor_scalar_max(hr[:], HL[:], 0.0)
            h = sb.tile([OSH, B], F32)
            nc.vector.tensor_mul(h[:], hr[:], d[:])
            nc.sync.dma_start(h_sh[:, :], h[:])

            # y_part[c,b] = sum_{o in shard} head[o,c]*h[o,b]  (fp32 PE)
            Y = ps.tile([C, B], F32)
            nc.tensor.matmul(Y[:], hdt[:, :], h[:], start=True, stop=True)
            yo = sb.tile([C, B], F32)
            nc.vector.tensor_copy(yo[:], Y[:])
            nc.sync.dma_start(y_part[:, :], yo[:])
    nc.compile()
    return nc


def _chunk_major(mat_t: np.ndarray) -> np.ndarray:
    """[1024, cols] -> [128, KCH*cols]: row block k lands at column
    offset k*cols, so partition dim is 128 and chunk k is a column
    slice."""
    rows, cols = mat_t.shape
    assert rows == KCH * 128
    return np.ascontiguousarray(
        mat_t.reshape(KCH, 128, cols).transpose(1, 0, 2).reshape(128, KCH * cols)
    )


def _split_hi_lo_packed(arr_f32: np.ndarray):
    hi = arr_f32.astype(BF16_NP)
    lo = (arr_f32 - hi.astype(np.float32)).astype(BF16_NP)
    return np.ascontiguousarray(np.concatenate([hi, lo], axis=1))


def _run_layer(nc, act, v, fc, head):
    """act: [B, 1024] layer input. Returns (h [B, HID], y_partial [C, B])."""
    xT = _chunk_major(np.ascontiguousarray(act.T.astype(np.float32)))
    xhl = _split_hi_lo_packed(xT)
    in_maps = []
    for c in range(N_CORES):
        sl = slice(c * OSH, (c + 1) * OSH)
        fT = _chunk_major(np.ascontiguousarray(fc[sl].T.astype(np.float32)))
        in_maps.append({
            "xTf": xT,
            "xhl": xhl,
            "vT": _chunk_major(np.ascontiguousarray(v[sl].T)).astype(BF16_NP),
            "fhl": _split_hi_lo_packed(fT),
            "hdT": np.ascontiguousarray(head[:, sl].T.astype(np.float32)),
        })
    kwargs = {}
    if PROFILE["enable"]:
        kwargs = {"trace": True, **PROFILE["trace_kwargs"]}
    res = run_bass_kernel_spmd(nc, in_maps, core_ids=list(range(N_CORES)),
                               **kwargs)
    if PROFILE["enable"]:
        PROFILE["runs"].append(res)
    hT = np.concatenate([res.results[c]["h_sh"] for c in range(N_CORES)],
                        axis=0)                      # [HID, B]
    y = np.zeros((C, B), np.float32)
    for c in range(N_CORES):
        y += res.results[c]["y_part"]
    return np.ascontiguousarray(hT.T), y


def kernel(x, v0, fc0, head0, v1, fc1, head1):
    nc = _CACHE.get("nc")
    if nc is None:
        nc = _CACHE["nc"] = _build()
    h0, yA = _run_layer(nc, np.asarray(x, np.float32), v0, fc0, head0)
    _, yB = _run_layer(nc, h0, v1, fc1, head1)
    return np.ascontiguousarray((yA + yB).T).astype(np.float32)


# revision 14
# speedup vs baseline: 1.0167x; 1.0167x over previous
"""HarsanyiNet forward on 8 TRN2 NeuronCores (Bass/Tile).

Model (reference):
    harsanyi_block(x, v, fc):
        m = (v > 0)                                    # [O, I] mask
        delta = prod_i [ tanh(g*|x_i|) if m else 1 ]   # [B, O]
        h = relu((x @ (fc*m).T) * delta)
    y = h0 @ head0.T + h1 @ head1.T   (two blocks, h0 feeds block 1)

Key algebraic moves:
  * The [B, O, I] masked product becomes a matmul in log space:
        delta = exp(L @ m.T),  L[b,i] = log(tanh(g*|x[b,i]|))
    with log(tanh(y)) = ln(1-z) - ln(1+z), z = exp(-2*g*y), so the
    whole transcendental chain is {abs, exp, ln} — all in ONE ScalarE
    table set (natural_log_exp_and_others) -> a single table load.
  * Matmuls run on the bf16 PE path (4x the fp32 rate) with hi/lo
    split operands for fp32-grade accuracy.  The mask m is exact in
    bf16; fc and x are split on the host (w_hi = m*bf16_hi(fc) is
    exact because masking by 0/1 commutes with rounding); L is split
    on-device.  The z<=1-2^-24 clamp keeps Ln inputs positive, so
    every intermediate stays finite.
  * The serial DMA->abs->exp->min->ln->ln->sub->split chain is
    pipelined in two column halves so ScalarE and VectorE stages of
    half 0 overlap half 1.

Sharding: the output-hidden dim is split across the 8 cores, so each
core reads only 1/8 of v/fc per layer (~0.8 MB/core/launch instead of
16.4 MB replicated).  Layer 1 needs the full h0, which is bounced
through the host between two launches of the SAME compiled program
(an on-device AllGather costs ~80us in this environment, the host
bounce costs zero device time).  Partial head outputs are summed on
the host.

Layout: on-device tensors are feature-major [feature, batch]; the
1024-long feature dims are pre-split on the host into 8 chunk-major
blocks of 128 partitions, so every DMA is one dense [128, N] transfer
and every matmul operand slice is a natural column block.
"""
import sys

import numpy as np

sys.path.insert(0, "/opt/trn_rl_repo")

import ml_dtypes  # noqa: E402

from concourse import bacc, mybir, tile  # noqa: E402
from concourse.alu_op_type import AluOpType  # noqa: E402
from concourse.bass_utils import run_bass_kernel_spmd  # noqa: E402
from concourse.tile_rust import add_dep_helper  # noqa: E402


def _order(after, before, why):
    """Order-only scheduling edge: `after` runs after `before`."""
    add_dep_helper(getattr(after, "ins", after), getattr(before, "ins", before),
                   sync=False, reason=why)

B, NIN, HID, C = 64, 1024, 1024, 10
GAMMA = 100.0
N_CORES = 8
OSH = HID // N_CORES        # output-hidden rows per core (128)
KCH = NIN // 128            # contraction chunks (8)
KB = KCH * B                # activation columns, chunk-major (512)
KO = KCH * OSH              # weight columns, chunk-major (1024)
NH = 2                      # pipeline halves for the L chain
HB = KB // NH               # columns per half (256)
HCH = KCH // NH             # chunks per half (4)
# Upper clamp for z = exp(-2g|x|): keeps 1-z >= 2^-24 so Ln never sees 0
# (the reference's exact-zero delta becomes exp(-16.6)~3e-8 per factor,
# far below the output's scale).
LCLAMP = -30000.0
F32 = mybir.dt.float32
BF16 = mybir.dt.bfloat16
BF16_NP = ml_dtypes.bfloat16

PROFILE = {"enable": False, "trace_kwargs": {}, "runs": []}
_CACHE = {}


def _force_act_table_set(target="natural_log_exp_and_others"):
    """Make the act-table-load pass place every activation in `target`
    (it otherwise picks the first set per function, costing one ~2.7us
    table switch per transition Exp->Ln->Exp).  Indices of the table
    list are act_func_set_ids, so ordering is preserved and all other
    sets are emptied."""
    import concourse.bacc as bacc_mod
    from concourse.hw_specs import get_activation_tables as real_tabs

    def patched(arch):
        tabs = real_tabs(arch)
        return {name: (funcs if name == target else set())
                for name, funcs in tabs.items()}

    bacc_mod.get_activation_tables = patched


def _build():
    _force_act_table_set()
    nc = bacc.Bacc("TRN2", target_bir_lowering=False, debug=False,
                   num_devices=N_CORES, enable_asserts=False)
    xTf = nc.declare_dram_parameter("xTf", [128, KB], F32, isOutput=False)
    # bf16 hi/lo pairs packed side by side: [hi | lo]
    xhl = nc.declare_dram_parameter("xhl", [128, 2 * KB], BF16, isOutput=False)
    vT = nc.declare_dram_parameter("vT", [128, KO], BF16, isOutput=False)
    fhl = nc.declare_dram_parameter("fhl", [128, 2 * KO], BF16, isOutput=False)
    hdT = nc.declare_dram_parameter("hdT", [OSH, C], F32, isOutput=False)
    h_sh = nc.declare_dram_parameter("h_sh", [OSH, B], F32, isOutput=True)
    y_part = nc.declare_dram_parameter("y_part", [C, B], F32, isOutput=True)
    Act = mybir.ActivationFunctionType

    with tile.TileContext(nc) as tc:
        with (
            tc.tile_pool(name="sb", bufs=1) as sb,
            tc.tile_pool(name="ps", bufs=1, space="PSUM") as ps,
        ):
            # x (f32) arrives in NH column-halves so the L chain can start
            # on half 0 while half 1 is still in flight.
            xf = sb.tile([128, KB], F32)
            for hf in range(NH):
                nc.sync.dma_start(xf[:, hf * HB:(hf + 1) * HB],
                                  xTf[:, hf * HB:(hf + 1) * HB])
            vt = sb.tile([128, KO], BF16)
            nc.sync.dma_start(vt[:], vT[:, :])
            fb = sb.tile([128, 2 * KO], BF16)
            nc.sync.dma_start(fb[:], fhl[:, :])
            xb = sb.tile([128, 2 * KB], BF16)
            nc.sync.dma_start(xb[:], xhl[:, :])
            hdt = sb.tile([OSH, C], F32)
            nc.sync.dma_start(hdt[:], hdT[:, :])

            # L = log(tanh(g*|x|)) = ln(1-z) - ln(1+z), z = exp(-2g|x|),
            # pipelined over NH column halves.  The small negative bias on
            # the Exp input keeps z strictly below 1 (so Ln(1-z) is finite
            # for x = 0); the -30000 clamp catches -inf if the LUT rounds
            # z up to 1 anyway.
            a = sb.tile([128, KB], F32)
            z = sb.tile([128, KB], F32)
            p = sb.tile([128, KB], F32)
            q = sb.tile([128, KB], F32)
            Lh = sb.tile([128, KB], BF16)
            Ll = sb.tile([128, KB], BF16)
            L = sb.tile([128, KB], F32)
            eps = sb.tile([128, 1], F32)
            nc.vector.memset(eps[:], -1e-6)
            m = sb.tile([128, KO], BF16)
            w = sb.tile([128, 2 * KO], BF16)
            S = ps.tile([OSH, B], F32)
            HL = ps.tile([OSH, B], F32)
            n_s = 2 * KCH
            i_s = 0
            s_last = None

            def s_mms(hf):
                nonlocal i_s, s_last
                for k in range(hf * HCH, (hf + 1) * HCH):
                    osl = slice(k * OSH, (k + 1) * OSH)
                    bsl = slice(k * B, (k + 1) * B)
                    for rhs in (Lh, Ll):
                        s_last = nc.tensor.matmul(S[:], m[:, osl],
                                                  rhs[:, bsl],
                                                  start=(i_s == 0),
                                                  stop=(i_s == n_s - 1))
                        i_s += 1

            subl = None
            for hf in range(NH):
                cs = slice(hf * HB, (hf + 1) * HB)
                nc.vector.scalar_tensor_tensor(a[:, cs], xf[:, cs], -1.0,
                                               xf[:, cs],
                                               op0=AluOpType.mult,
                                               op1=AluOpType.max)
                nc.scalar.activation(z[:, cs], a[:, cs], Act.Exp,
                                     scale=-2.0 * GAMMA, bias=eps[:])
                nc.scalar.activation(p[:, cs], z[:, cs], Act.Ln,
                                     bias=1.0, scale=-1.0)
                nc.scalar.activation(q[:, cs], z[:, cs], Act.Ln,
                                     bias=1.0, scale=1.0)
                if hf == 0:
                    # m = (v > 0) as 0/1 (v is exactly +-1): slot into the
                    # DVE stream while ScalarE works on ln, right before
                    # the first L ops so S matmuls can start early.
                    nc.vector.tensor_scalar_max(m[:], vt[:], 0.0)
                nc.vector.scalar_tensor_tensor(L[:, cs], p[:, cs], LCLAMP,
                                               q[:, cs],
                                               op0=AluOpType.max,
                                               op1=AluOpType.subtract)
                nc.vector.tensor_copy(Lh[:, cs], L[:, cs])
                subl = nc.vector.tensor_sub(Ll[:, cs], L[:, cs], Lh[:, cs])
                s_mms(hf)

            # w = fc * m.  The scheduler's cost model doesn't see DMA
            # latency and would hoist these (blocked on the fc DMA) ahead
            # of the ready L-chain ops on the in-order DVE; pin them after
            # the last L split.
            w0 = nc.vector.tensor_mul(w[:, :KO], m[:], fb[:, :KO])
            w1 = nc.vector.tensor_mul(w[:, KO:], m[:], fb[:, KO:])
            _order(w0, subl, "w after L splits (DVE head-of-line)")
            _order(w1, w0, "w_lo after w_hi")

            # HL matmuls last: they wait on the (late) fc DMA anyway, and
            # keeping them off the in-order PE queue lets S finish early.
            n_hl = 3 * KCH
            i_hl = 0
            for k in range(KCH):
                osl = slice(k * OSH, (k + 1) * OSH)
                osl_lo = slice(KO + k * OSH, KO + (k + 1) * OSH)
                bsl = slice(k * B, (k + 1) * B)
                bsl_lo = slice(KB + k * B, KB + (k + 1) * B)
                # HL += w_hi.T x_hi + w_hi.T x_lo + w_lo.T x_hi
                for lsl, rsl in ((osl, bsl), (osl, bsl_lo), (osl_lo, bsl)):
                    mm = nc.tensor.matmul(HL[:], w[:, lsl], xb[:, rsl],
                                          start=(i_hl == 0),
                                          stop=(i_hl == n_hl - 1))
                    if i_hl == 0:
                        _order(mm, s_last, "HL matmuls after S matmuls (PE)")
                    i_hl += 1

            # h = relu(HL) * exp(S)   (= relu(HL*exp(S)) since exp(S) > 0;
            # the relu runs as soon as HL closes, in parallel with exp)
            d = sb.tile([OSH, B], F32)
            nc.scalar.activation(d[:], S[:], Act.Exp)
            hr = sb.tile([OSH, B], F32)
            nc.vector.tensor_scalar_max(hr[:], HL[:], 0.0)
            h = sb.tile([OSH, B], F32)
            nc.vector.tensor_mul(h[:], hr[:], d[:])
            nc.sync.dma_start(h_sh[:, :], h[:])

            # y_part[c,b] = sum_{o in shard} head[o,c]*h[o,b]  (fp32 PE)
            Y = ps.tile([C, B], F32)
            nc.tensor.matmul(Y[:], hdt[:, :], h[:], start=True, stop=True)
            yo = sb.tile([C, B], F32)
            nc.vector.tensor_copy(yo[:], Y[:])
            nc.sync.dma_start(y_part[:, :], yo[:])
    nc.compile()
    return nc


def _chunk_major(mat_t: np.ndarray) -> np.ndarray:
    """[1024, cols] -> [128, KCH*cols]: row block k lands at column
    offset k*cols, so partition dim is 128 and chunk k is a column
    slice."""
    rows, cols = mat_t.shape
    assert rows == KCH * 128
    return np.ascontiguousarray(
        mat_t.reshape(KCH, 128, cols).transpose(1, 0, 2).reshape(128, KCH * cols)
    )


def _split_hi_lo_packed(arr_f32: np.ndarray):
    hi = arr_f32.astype(BF16_NP)
    lo = (arr_f32 - hi.astype(np.float32)).astype(BF16_NP)
    return np.ascontiguousarray(np.concatenate([hi, lo], axis=1))


def _run_layer(nc, act, v, fc, head):
    """act: [B, 1024] layer input. Returns (h [B, HID], y_partial [C, B])."""
    xT = _chunk_major(np.ascontiguousarray(act.T.astype(np.float32)))
    xhl = _split_hi_lo_packed(xT)
    in_maps = []
    for c in range(N_CORES):
        sl = slice(c * OSH, (c + 1) * OSH)
        fT = _chunk_major(np.ascontiguousarray(fc[sl].T.astype(np.float32)))
        in_maps.append({
            "xTf": xT,
            "xhl": xhl,
            "vT": _chunk_major(np.ascontiguousarray(v[sl].T)).astype(BF16_NP),
            "fhl": _split_hi_lo_packed(fT),
            "hdT": np.ascontiguousarray(head[:, sl].T.astype(np.float32)),
        })
    kwargs = {}
    if PROFILE["enable"]:
        kwargs = {"trace": True, **PROFILE["trace_kwargs"]}
    res = run_bass_kernel_spmd(nc, in_maps, core_ids=list(range(N_CORES)),
                               **kwargs)
    if PROFILE["enable"]:
        PROFILE["runs"].append(res)
    hT = np.concatenate([res.results[c]["h_sh"] for c in range(N_CORES)],
                        axis=0)                      # [HID, B]
    y = np.zeros((C, B), np.float32)
    for c in range(N_CORES):
        y += res.results[c]["y_part"]
    return np.ascontiguousarray(hT.T), y


def kernel(x, v0, fc0, head0, v1, fc1, head1):
    nc = _CACHE.get("nc")
    if nc is None:
        nc = _CACHE["nc"] = _build()
    h0, yA = _run_layer(nc, np.asarray(x, np.float32), v0, fc0, head0)
    _, yB = _run_layer(nc, h0, v1, fc1, head1)
    return np.ascontiguousarray((yA + yB).T).astype(np.float32)
